# revision 35
# baseline (speedup 1.0000x reference)
"""Multi-head attention (B=2, S=2048, D=1024, H=16) on 8 TRN2 NeuronCores.

Sharding: tensor-parallel over heads x data-parallel over batch.
Core c handles batch b = c // 4 and head group g = c % 4 (4 heads each).
Each core computes its 4 heads' q/k/v projections, attention, and the
partial output projection against its slice of Wo; the host sums the 4
partials per batch element.

Per-core kernel layout:
  - inputs: xT [1024, 2048] (= x[b].T), wq/wk/wv [1024, 256] (= W[rows].T),
    wo [256, 1024] (= Wo[:, cols].T)
  - QT/KT/VT computed transposed ([head-feat, seq]) so the Dh-contraction
    of q@k^T has its contraction dim on partitions.
  - scores are computed transposed ([keys, q]) for a head PAIR into one
    2-bank psum tile; one wide exp via ACT (scale folded); attn @ v
    contracts keys on partitions; columns 64:128 of the v operand hold
    ones so the same matmul emits softmax row-sums replicated across 64
    psum rows (partition-broadcast APs are illegal on DVE, replicating in
    the matmul is free).

fp16 streaming: matmul operands are fp16 (1 cyc/row on the PE vs ~1.6 for
f32r, half the LDWEIGHTS and SBUF cost) while every accumulation stays
f32 in PSUM. Value ranges fit fp16 comfortably (|q|,|k| ~ N(0,1),
exp(scores*scale) <= ~e^7; fp16 max is 65504).
"""

import numpy as np

B, S, D, H, DH = 2, 2048, 1024, 16, 64
NCORES = 8
GROUPS = 4  # head groups; 4 heads = 256 features per core
M = 256  # head features per core
SCALE = 0.125  # 1/sqrt(64)

# stream dtypes per matmul group: "f32r", "bf16", or "fp16"
CFG = {
    "proj": "fp16",   # xT, wq/wk/wv
    "scores": "fp16",  # QT, KT
    "av": "fp16",      # VA, exp tiles
    "wo": "fp16",      # OT, wo
}

_compiled = None


def _dt(mybir, name):
    return {"f32r": mybir.dt.float32r, "bf16": mybir.dt.bfloat16,
            "fp16": mybir.dt.float16}[name]


def _np_dt(name):
    if name == "bf16":
        import ml_dtypes
        return ml_dtypes.bfloat16
    if name == "fp16":
        return np.float16
    return np.float32


def _build_module():
    import concourse.mybir as mybir
    import concourse.tile as tile
    from concourse import bacc

    f32 = mybir.dt.float32
    in_dt = _dt(mybir, CFG["proj"])
    wo_dt = _dt(mybir, CFG["wo"])
    nc = bacc.Bacc("TRN2", target_bir_lowering=False, debug=False,
                   num_devices=NCORES)
    xT = nc.dram_tensor("xT", [D, S], in_dt, kind="ExternalInput").ap()
    wq = nc.dram_tensor("wq", [D, M], in_dt, kind="ExternalInput").ap()
    wk = nc.dram_tensor("wk", [D, M], in_dt, kind="ExternalInput").ap()
    wv = nc.dram_tensor("wv", [D, M], in_dt, kind="ExternalInput").ap()
    wo = nc.dram_tensor("wo", [M, D], wo_dt, kind="ExternalInput").ap()
    out = nc.dram_tensor("out", [S, D], f32, kind="ExternalOutput").ap()

    with tile.TileContext(nc) as tc:
        _kernel_body(tc, out, xT, wq, wk, wv, wo)
    nc.compile()
    return nc


def _kernel_body(tc, out, xT, wq, wk, wv, wo):
    from contextlib import ExitStack

    import concourse.mybir as mybir
    from concourse.masks import make_identity

    nc = tc.nc
    f32 = mybir.dt.float32
    f32r = mybir.dt.float32r
    sc_dt = _dt(mybir, CFG["scores"])
    av_dt = _dt(mybir, CFG["av"])
    wo_dt = _dt(mybir, CFG["wo"])
    AF = mybir.ActivationFunctionType

    P = 128
    NKT = D // P   # 8 k-tiles in the projection contraction
    NPT = M // P   # 2 partition-tiles of head features
    SKT = S // P   # 16 key tiles
    QC = 512       # q chunk (psum bank width in f32)
    NQC = S // QC  # 4
    HPC = 4        # heads per core

    with ExitStack() as ctx:
        const = ctx.enter_context(tc.tile_pool(name="const", bufs=1))
        big = ctx.enter_context(tc.tile_pool(name="big", bufs=1))
        wpool = ctx.enter_context(tc.tile_pool(name="w", bufs=1))
        work = ctx.enter_context(tc.tile_pool(name="work", bufs=2))
        exp_pool = ctx.enter_context(tc.tile_pool(name="exp", bufs=6))
        small = ctx.enter_context(tc.tile_pool(name="small", bufs=2))
        # PSUM budget (8 banks): psA 2 + psS 2x2 + psO 2x1 = 8
        psum_big = ctx.enter_context(tc.tile_pool(name="psA", bufs=2, space="PSUM"))
        psum_s = ctx.enter_context(tc.tile_pool(name="psS", bufs=2, space="PSUM"))
        psum_o = ctx.enter_context(tc.tile_pool(name="psO", bufs=1, space="PSUM"))

        ident_f = const.tile([P, P], f32)
        make_identity(nc, ident_f)
        ident = const.tile([P, P], f32r, tag="ident_r")
        nc.vector.tensor_copy(ident[:], ident_f[:])

        # warm the PE clock (HAM) during the input DMA head: ~40 dummy
        # matmuls on the identity keep the activity monitor busy so the
        # real projections start at 2.4GHz instead of 1.2
        warm_ps = psum_big.tile([P, P], f32, tag="ps_big")
        for _ in range(40):
            nc.tensor.matmul(warm_ps[:], ident[:], ident[:],
                             start=True, stop=True)

        QT = big.tile([P, NPT, S], sc_dt, tag="QT")
        KT = big.tile([P, NPT, S], sc_dt, tag="KT")
        # VT is fully consumed by the transpose phase before OT is first
        # written, so they share a buffer slot.
        VT = big.tile([P, NPT, S], f32r, tag="VT_OT")
        proj_dst = {"q": QT, "k": KT, "v": VT}
        wo_sb = wpool.tile([P, NPT, D], wo_dt, tag="wo")
        nc.sync.dma_start(wo_sb[:], wo.rearrange("(pt p) n -> p pt n", p=P))

        # xT and the q/k/v weights are only needed for the projections;
        # scope them so their SBUF is reusable afterwards.
        with tc.tile_pool(name="projin", bufs=1) as projin:
            # weights on the gpsimd (SWDGE) queue, bulk xT on the sync
            # (HWDGE) queue so they transfer in parallel and the first
            # projection matmul isn't stuck behind the bulk transfer
            w_sb = {}
            for name, w in (("k", wk), ("v", wv), ("q", wq)):
                t = projin.tile([P, NKT, M], w.dtype, tag=f"w{name}")
                nc.gpsimd.dma_start(t[:], w.rearrange("(kt p) m -> p kt m", p=P))
                w_sb[name] = t

            xT_sb = projin.tile([P, NKT, S], xT.dtype, tag="xT")
            xT_r = xT.rearrange("(kt p) s -> p kt s", p=P)
            for c in range(NQC):
                for kh in range(4 if c == 0 else 2):
                    n = NKT // (4 if c == 0 else 2)
                    nc.sync.dma_start(
                        xT_sb[:, kh * n:(kh + 1) * n, c * QC:(c + 1) * QC],
                        xT_r[:, kh * n:(kh + 1) * n, c * QC:(c + 1) * QC])

            # --- q/k/v projections: PT[f, s] = sum_d w[d, f] * xT[d, s].
            # pt-outer so head pair 0 is fully projected (and transposed)
            # before pair 1 starts; pair 0's attention overlaps pair 1's
            # projections. ---
            VA = big.tile([P, HPC, SKT, P], av_dt, tag="VA")
            ones_f = const.tile([P, 64], f32, tag="ones")
            nc.any.memset(ones_f[:], 1.0)
            for h in range(HPC):
                for st in range(SKT):
                    nc.vector.tensor_copy(VA[:, h, st, 64:128], ones_f[:])
            for pt in range(NPT):
                for name in ("k", "v", "q"):
                    dst = proj_dst[name]
                    for c in range(NQC):
                        ps = psum_big.tile([P, QC], f32, tag="ps_big")
                        for kt in range(NKT):
                            nc.tensor.matmul(
                                ps[:],
                                w_sb[name][:, kt, pt * P:(pt + 1) * P],
                                xT_sb[:, kt, c * QC:(c + 1) * QC],
                                start=(kt == 0), stop=(kt == NKT - 1),
                            )
                        nc.any.tensor_copy(dst[:, pt, c * QC:(c + 1) * QC], ps[:])
                # V back to natural layout + ones block for softmax sums
                for st in range(SKT):
                    pst = psum_big.tile([P, P], f32r, tag="ps_big")
                    nc.tensor.transpose(pst[:], VT[:, pt, st * P:(st + 1) * P],
                                        ident)
                    nc.any.tensor_copy(VA[:, 2 * pt, st, 0:64], pst[:, 0:64])
                    nc.any.tensor_copy(VA[:, 2 * pt + 1, st, 0:64],
                                       pst[:, 64:128])

        # 2-byte OT fits alongside VT; f32r must reuse VT's slot (SBUF)
        OT = big.tile([P, NPT, S], wo_dt,
                      tag="VT_OT" if wo_dt == f32r else "OT")

        # --- attention: a head pair per iteration; both heads' transposed
        # score tiles land in one 2-bank psum tile, one wide exp serves both,
        # and the PE always has an independent chain while ACT runs ---
        # matmul psum output must be f32; one key-tile (head pair) per fill
        KG, sc_ps_dt = 1, f32
        for p in range(NPT):
            for c in range(NQC):
                cs = slice(c * QC, (c + 1) * QC)
                poA = psum_o.tile([P, QC], f32, tag="ps_oA")
                poB = psum_o.tile([P, QC], f32, tag="ps_oB")
                for k2 in range(SKT // KG):
                    pss = psum_s.tile([P, 2 * KG, QC], sc_ps_dt, tag="ps_s")
                    for j in range(KG):
                        kt = KG * k2 + j
                        ks = slice(kt * P, (kt + 1) * P)
                        nc.tensor.matmul(pss[:, 2 * j, :], KT[0:64, p, ks],
                                         QT[0:64, p, cs], start=True, stop=True)
                        nc.tensor.matmul(pss[:, 2 * j + 1, :],
                                         KT[64:128, p, ks],
                                         QT[64:128, p, cs], start=True,
                                         stop=True)
                    et = exp_pool.tile([P, 2 * KG, QC], av_dt, tag="exp")
                    nc.scalar.activation(et[:], pss[:], AF.Exp, scale=SCALE)
                    for j in range(KG):
                        kt = KG * k2 + j
                        nc.tensor.matmul(poA[:], VA[:, 2 * p, kt, :],
                                         et[:, 2 * j, :],
                                         start=(kt == 0), stop=(kt == SKT - 1))
                        nc.tensor.matmul(poB[:], VA[:, 2 * p + 1, kt, :],
                                         et[:, 2 * j + 1, :],
                                         start=(kt == 0), stop=(kt == SKT - 1))
                for r0, po in ((0, poA), (64, poB)):
                    # evacuate po in one copy so the psum bank frees early,
                    # then normalize from the SBUF copy off the critical path
                    pc = small.tile([P, QC], f32, tag="po_sb")
                    nc.vector.tensor_copy(pc[:], po[:])
                    sm = small.tile([64, QC], f32, tag="sums")
                    nc.vector.tensor_copy(sm[:], pc[64:128, :])
                    rb = small.tile([64, QC], f32, tag="recip")
                    nc.vector.reciprocal_approx_fast(rb[:], sm[:])
                    nc.vector.tensor_tensor(
                        OT[r0:r0 + 64, p, cs],
                        pc[0:64, :],
                        rb[:],
                        mybir.AluOpType.mult,
                    )

        # --- output projection: out[s, n] = sum_f OT[f, s] * wo[f, n] ---
        for qt in range(S // P):
            for nch in range(2):
                ps = psum_big.tile([P, 512], f32, tag="ps_big")
                for pt in range(NPT):
                    nc.tensor.matmul(
                        ps[:],
                        OT[:, pt, qt * P:(qt + 1) * P],
                        wo_sb[:, pt, nch * 512:(nch + 1) * 512],
                        start=(pt == 0), stop=(pt == NPT - 1),
                    )
                ot = work.tile([P, 512], f32, tag="outstage")
                nc.any.tensor_copy(ot[:], ps[:])
                nc.sync.dma_start(
                    out[qt * P:(qt + 1) * P, nch * 512:(nch + 1) * 512], ot[:])


def _in_maps(x, Wq, Wk, Wv, Wo):
    in_np = _np_dt(CFG["proj"])
    wo_np = _np_dt(CFG["wo"])
    x = np.asarray(x, dtype=np.float32)
    Wq = np.asarray(Wq, dtype=np.float32)
    Wk = np.asarray(Wk, dtype=np.float32)
    Wv = np.asarray(Wv, dtype=np.float32)
    Wo = np.asarray(Wo, dtype=np.float32)
    xT = [np.ascontiguousarray(x[b].T).astype(in_np) for b in range(B)]
    maps = []
    for c in range(NCORES):
        b, g = c // GROUPS, c % GROUPS
        rows = slice(g * M, (g + 1) * M)
        maps.append({
            "xT": xT[b],
            "wq": np.ascontiguousarray(Wq[rows, :].T).astype(in_np),
            "wk": np.ascontiguousarray(Wk[rows, :].T).astype(in_np),
            "wv": np.ascontiguousarray(Wv[rows, :].T).astype(in_np),
            "wo": np.ascontiguousarray(Wo[:, rows].T).astype(wo_np),
        })
    return maps


def kernel(x, Wq, Wk, Wv, Wo, _trace=False):
    global _compiled
    if _compiled is None:
        _compiled = _build_module()
    from concourse.bass_utils import run_bass_kernel_spmd

    res = run_bass_kernel_spmd(
        _compiled, _in_maps(x, Wq, Wk, Wv, Wo),
        core_ids=list(range(NCORES)), trace=_trace,
    )
    outs = [r["out"] for r in res.results]
    y = np.empty((B, S, D), np.float32)
    for b in range(B):
        y[b] = outs[4 * b] + outs[4 * b + 1] + outs[4 * b + 2] + outs[4 * b + 3]
    if _trace:
        kernel.last_results = res
    return y


# revision 37
# speedup vs baseline: 1.0081x; 1.0081x over previous
"""Multi-head attention (B=2, S=2048, D=1024, H=16) on 8 TRN2 NeuronCores.

Sharding: tensor-parallel over heads x data-parallel over batch.
Core c handles batch b = c // 4 and head group g = c % 4 (4 heads each).
Each core computes its 4 heads' q/k/v projections, attention, and the
partial output projection against its slice of Wo; the host sums the 4
partials per batch element.

Per-core kernel layout:
  - inputs: xT [1024, 2048] (= x[b].T), wq/wk/wv [1024, 256] (= W[rows].T),
    wo [256, 1024] (= Wo[:, cols].T)
  - QT/KT/VT computed transposed ([head-feat, seq]) so the Dh-contraction
    of q@k^T has its contraction dim on partitions.
  - scores are computed transposed ([keys, q]) for a head PAIR into one
    2-bank psum tile; one wide exp via ACT (scale folded); attn @ v
    contracts keys on partitions; columns 64:128 of the v operand hold
    ones so the same matmul emits softmax row-sums replicated across 64
    psum rows (partition-broadcast APs are illegal on DVE, replicating in
    the matmul is free).

fp16 streaming: matmul operands are fp16 (1 cyc/row on the PE vs ~1.6 for
f32r, half the LDWEIGHTS and SBUF cost) while every accumulation stays
f32 in PSUM. Value ranges fit fp16 comfortably (|q|,|k| ~ N(0,1),
exp(scores*scale) <= ~e^7; fp16 max is 65504).
"""

import numpy as np

B, S, D, H, DH = 2, 2048, 1024, 16, 64
NCORES = 8
GROUPS = 4  # head groups; 4 heads = 256 features per core
M = 256  # head features per core
SCALE = 0.125  # 1/sqrt(64)

# stream dtypes per matmul group: "f32r", "bf16", or "fp16"
CFG = {
    "proj": "fp16",   # xT, wq/wk/wv
    "scores": "fp16",  # QT, KT
    "av": "fp16",      # VA, exp tiles
    "wo": "fp16",      # OT, wo
}

_compiled = None


def _dt(mybir, name):
    return {"f32r": mybir.dt.float32r, "bf16": mybir.dt.bfloat16,
            "fp16": mybir.dt.float16}[name]


def _np_dt(name):
    if name == "bf16":
        import ml_dtypes
        return ml_dtypes.bfloat16
    if name == "fp16":
        return np.float16
    return np.float32


def _build_module():
    import concourse.mybir as mybir
    import concourse.tile as tile
    from concourse import bacc

    f32 = mybir.dt.float32
    in_dt = _dt(mybir, CFG["proj"])
    wo_dt = _dt(mybir, CFG["wo"])
    nc = bacc.Bacc("TRN2", target_bir_lowering=False, debug=False,
                   num_devices=NCORES)
    xT = nc.dram_tensor("xT", [D, S], in_dt, kind="ExternalInput").ap()
    wq = nc.dram_tensor("wq", [D, M], in_dt, kind="ExternalInput").ap()
    wk = nc.dram_tensor("wk", [D, M], in_dt, kind="ExternalInput").ap()
    wv = nc.dram_tensor("wv", [D, M], in_dt, kind="ExternalInput").ap()
    wo = nc.dram_tensor("wo", [M, D], wo_dt, kind="ExternalInput").ap()
    out = nc.dram_tensor("out", [S, D], f32, kind="ExternalOutput").ap()

    with tile.TileContext(nc) as tc:
        _kernel_body(tc, out, xT, wq, wk, wv, wo)
    nc.compile()
    return nc


def _kernel_body(tc, out, xT, wq, wk, wv, wo):
    from contextlib import ExitStack

    import concourse.mybir as mybir
    from concourse.masks import make_identity

    nc = tc.nc
    f32 = mybir.dt.float32
    f32r = mybir.dt.float32r
    sc_dt = _dt(mybir, CFG["scores"])
    av_dt = _dt(mybir, CFG["av"])
    wo_dt = _dt(mybir, CFG["wo"])
    AF = mybir.ActivationFunctionType

    P = 128
    NKT = D // P   # 8 k-tiles in the projection contraction
    NPT = M // P   # 2 partition-tiles of head features
    SKT = S // P   # 16 key tiles
    QC = 512       # q chunk (psum bank width in f32)
    NQC = S // QC  # 4
    HPC = 4        # heads per core

    with ExitStack() as ctx:
        const = ctx.enter_context(tc.tile_pool(name="const", bufs=1))
        big = ctx.enter_context(tc.tile_pool(name="big", bufs=1))
        wpool = ctx.enter_context(tc.tile_pool(name="w", bufs=1))
        work = ctx.enter_context(tc.tile_pool(name="work", bufs=2))
        exp_pool = ctx.enter_context(tc.tile_pool(name="exp", bufs=6))
        small = ctx.enter_context(tc.tile_pool(name="small", bufs=2))
        # PSUM budget (8 banks): psA 2 + psS 2x2 + psO 2x1 = 8
        psum_big = ctx.enter_context(tc.tile_pool(name="psA", bufs=2, space="PSUM"))
        psum_s = ctx.enter_context(tc.tile_pool(name="psS", bufs=2, space="PSUM"))
        psum_o = ctx.enter_context(tc.tile_pool(name="psO", bufs=1, space="PSUM"))

        ident_f = const.tile([P, P], f32)
        make_identity(nc, ident_f)
        ident = const.tile([P, P], f32r, tag="ident_r")
        nc.vector.tensor_copy(ident[:], ident_f[:])

        # warm the PE clock (HAM) during the input DMA head: ~40 dummy
        # matmuls on the identity keep the activity monitor busy so the
        # real projections start at 2.4GHz instead of 1.2
        warm_ps = psum_big.tile([P, P], f32, tag="ps_big")
        for _ in range(40):
            nc.tensor.matmul(warm_ps[:], ident[:], ident[:],
                             start=True, stop=True)

        QT = big.tile([P, NPT, S], sc_dt, tag="QT")
        KT = big.tile([P, NPT, S], sc_dt, tag="KT")
        # VT is fully consumed by the transpose phase before OT is first
        # written, so they share a buffer slot.
        VT = big.tile([P, NPT, S], f32r, tag="VT_OT")
        proj_dst = {"q": QT, "k": KT, "v": VT}
        wo_sb = wpool.tile([P, NPT, D], wo_dt, tag="wo")
        nc.sync.dma_start(wo_sb[:], wo.rearrange("(pt p) n -> p pt n", p=P))

        # xT and the q/k/v weights are only needed for the projections;
        # scope them so their SBUF is reusable afterwards.
        with tc.tile_pool(name="projin", bufs=1) as projin:
            # weights on the gpsimd (SWDGE) queue, bulk xT on the sync
            # (HWDGE) queue so they transfer in parallel and the first
            # projection matmul isn't stuck behind the bulk transfer
            w_sb = {}
            for name, w in (("k", wk), ("v", wv), ("q", wq)):
                t = projin.tile([P, NKT, M], w.dtype, tag=f"w{name}")
                nc.gpsimd.dma_start(t[:], w.rearrange("(kt p) m -> p kt m", p=P))
                w_sb[name] = t

            xT_sb = projin.tile([P, NKT, S], xT.dtype, tag="xT")
            xT_r = xT.rearrange("(kt p) s -> p kt s", p=P)
            for c in range(NQC):
                for kh in range(4 if c == 0 else 2):
                    n = NKT // (4 if c == 0 else 2)
                    nc.sync.dma_start(
                        xT_sb[:, kh * n:(kh + 1) * n, c * QC:(c + 1) * QC],
                        xT_r[:, kh * n:(kh + 1) * n, c * QC:(c + 1) * QC])

            # --- q/k/v projections: PT[f, s] = sum_d w[d, f] * xT[d, s].
            # pt-outer so head pair 0 is fully projected (and transposed)
            # before pair 1 starts; pair 0's attention overlaps pair 1's
            # projections. ---
            VA = big.tile([P, HPC, SKT, P], av_dt, tag="VA")
            ones_f = const.tile([P, 64], f32, tag="ones")
            nc.any.memset(ones_f[:], 1.0)
            for h in range(HPC):
                for st in range(SKT):
                    nc.vector.tensor_copy(VA[:, h, st, 64:128], ones_f[:])
            for pt in range(NPT):
                for name in ("k", "v", "q"):
                    dst = proj_dst[name]
                    for c in range(NQC):
                        ps = psum_big.tile([P, QC], f32, tag="ps_big")
                        for kt in range(NKT):
                            nc.tensor.matmul(
                                ps[:],
                                w_sb[name][:, kt, pt * P:(pt + 1) * P],
                                xT_sb[:, kt, c * QC:(c + 1) * QC],
                                start=(kt == 0), stop=(kt == NKT - 1),
                            )
                        nc.any.tensor_copy(dst[:, pt, c * QC:(c + 1) * QC], ps[:])
                # V back to natural layout + ones block for softmax sums
                for st in range(SKT):
                    pst = psum_big.tile([P, P], f32r, tag="ps_big")
                    nc.tensor.transpose(pst[:], VT[:, pt, st * P:(st + 1) * P],
                                        ident)
                    nc.any.tensor_copy(VA[:, 2 * pt, st, 0:64], pst[:, 0:64])
                    nc.any.tensor_copy(VA[:, 2 * pt + 1, st, 0:64],
                                       pst[:, 64:128])

        # 2-byte OT fits alongside VT; f32r must reuse VT's slot (SBUF)
        OT = big.tile([P, NPT, S], wo_dt,
                      tag="VT_OT" if wo_dt == f32r else "OT")

        # --- attention: a head pair per iteration; both heads' transposed
        # score tiles land in one 2-bank psum tile, one wide exp serves both,
        # and the PE always has an independent chain while ACT runs ---
        # matmul psum output must be f32; one key-tile (head pair) per fill
        KG, sc_ps_dt = 1, f32
        for p in range(NPT):
            for c in range(NQC):
                cs = slice(c * QC, (c + 1) * QC)
                poA = psum_o.tile([P, QC], f32, tag="ps_oA")
                poB = psum_o.tile([P, QC], f32, tag="ps_oB")
                for k2 in range(SKT // KG):
                    pss = psum_s.tile([P, 2 * KG, QC], sc_ps_dt, tag="ps_s")
                    for j in range(KG):
                        kt = KG * k2 + j
                        ks = slice(kt * P, (kt + 1) * P)
                        nc.tensor.matmul(pss[:, 2 * j, :], KT[0:64, p, ks],
                                         QT[0:64, p, cs], start=True, stop=True)
                        nc.tensor.matmul(pss[:, 2 * j + 1, :],
                                         KT[64:128, p, ks],
                                         QT[64:128, p, cs], start=True,
                                         stop=True)
                    et = exp_pool.tile([P, 2 * KG, QC], av_dt, tag="exp")
                    nc.scalar.activation(et[:], pss[:], AF.Exp, scale=SCALE)
                    for j in range(KG):
                        kt = KG * k2 + j
                        nc.tensor.matmul(poA[:], VA[:, 2 * p, kt, :],
                                         et[:, 2 * j, :],
                                         start=(kt == 0), stop=(kt == SKT - 1))
                        nc.tensor.matmul(poB[:], VA[:, 2 * p + 1, kt, :],
                                         et[:, 2 * j + 1, :],
                                         start=(kt == 0), stop=(kt == SKT - 1))
                for r0, po in ((0, poA), (64, poB)):
                    # evacuate po in one copy so the psum bank frees early,
                    # then normalize from the SBUF copy off the critical path
                    pc = small.tile([P, QC], f32, tag="po_sb")
                    nc.vector.tensor_copy(pc[:], po[:])
                    sm = small.tile([64, QC], f32, tag="sums")
                    nc.vector.tensor_copy(sm[:], pc[64:128, :])
                    rb = small.tile([64, QC], f32, tag="recip")
                    nc.vector.reciprocal_approx_fast(rb[:], sm[:])
                    nc.vector.tensor_tensor(
                        OT[r0:r0 + 64, p, cs],
                        pc[0:64, :],
                        rb[:],
                        mybir.AluOpType.mult,
                    )

        # --- output projection: out[s, n] = sum_f OT[f, s] * wo[f, n] ---
        for qt in range(S // P):
            for nch in range(2):
                ps = psum_big.tile([P, 512], f32, tag="ps_big")
                for pt in range(NPT):
                    nc.tensor.matmul(
                        ps[:],
                        OT[:, pt, qt * P:(qt + 1) * P],
                        wo_sb[:, pt, nch * 512:(nch + 1) * 512],
                        start=(pt == 0), stop=(pt == NPT - 1),
                    )
                ot = work.tile([P, 512], f32, tag="outstage")
                nc.any.tensor_copy(ot[:], ps[:])
                nc.sync.dma_start(
                    out[qt * P:(qt + 1) * P, nch * 512:(nch + 1) * 512], ot[:])


def _in_maps(x, Wq, Wk, Wv, Wo):
    in_np = _np_dt(CFG["proj"])
    wo_np = _np_dt(CFG["wo"])
    x = np.asarray(x, dtype=np.float32)
    Wq = np.asarray(Wq, dtype=np.float32)
    Wk = np.asarray(Wk, dtype=np.float32)
    Wv = np.asarray(Wv, dtype=np.float32)
    Wo = np.asarray(Wo, dtype=np.float32)
    xT = [np.ascontiguousarray(x[b].T).astype(in_np) for b in range(B)]
    maps = []
    for c in range(NCORES):
        b, g = c // GROUPS, c % GROUPS
        rows = slice(g * M, (g + 1) * M)
        maps.append({
            "xT": xT[b],
            "wq": np.ascontiguousarray(Wq[rows, :].T).astype(in_np),
            "wk": np.ascontiguousarray(Wk[rows, :].T).astype(in_np),
            "wv": np.ascontiguousarray(Wv[rows, :].T).astype(in_np),
            "wo": np.ascontiguousarray(Wo[:, rows].T).astype(wo_np),
        })
    return maps


def kernel(x, Wq, Wk, Wv, Wo, _trace=False):
    global _compiled
    if _compiled is None:
        _compiled = _build_module()
    from concourse.bass_utils import run_bass_kernel_spmd

    res = run_bass_kernel_spmd(
        _compiled, _in_maps(x, Wq, Wk, Wv, Wo),
        core_ids=list(range(NCORES)), trace=_trace,
    )
    outs = [r["out"] for r in res.results]
    y = np.empty((B, S, D), np.float32)
    for b in range(B):
        y[b] = outs[4 * b] + outs[4 * b + 1] + outs[4 * b + 2] + outs[4 * b + 3]
    if _trace:
        kernel.last_results = res
    return y


# revision 39
# speedup vs baseline: 1.0142x; 1.0060x over previous
"""Multi-head attention (B=2, S=2048, D=1024, H=16) on 8 TRN2 NeuronCores.

Sharding: tensor-parallel over heads x data-parallel over batch.
Core c handles batch b = c // 4 and head group g = c % 4 (4 heads each).
Each core computes its 4 heads' q/k/v projections, attention, and the
partial output projection against its slice of Wo; the host sums the 4
partials per batch element.

Per-core kernel layout:
  - inputs: xT [1024, 2048] (= x[b].T), wq/wk/wv [1024, 256] (= W[rows].T),
    wo [256, 1024] (= Wo[:, cols].T)
  - QT/KT/VT computed transposed ([head-feat, seq]) so the Dh-contraction
    of q@k^T has its contraction dim on partitions.
  - scores are computed transposed ([keys, q]) for a head PAIR into one
    2-bank psum tile; one wide exp via ACT (scale folded); attn @ v
    contracts keys on partitions; columns 64:128 of the v operand hold
    ones so the same matmul emits softmax row-sums replicated across 64
    psum rows (partition-broadcast APs are illegal on DVE, replicating in
    the matmul is free).

fp16 streaming: matmul operands are fp16 (1 cyc/row on the PE vs ~1.6 for
f32r, half the LDWEIGHTS and SBUF cost) while every accumulation stays
f32 in PSUM. Value ranges fit fp16 comfortably (|q|,|k| ~ N(0,1),
exp(scores*scale) <= ~e^7; fp16 max is 65504).
"""

import numpy as np

B, S, D, H, DH = 2, 2048, 1024, 16, 64
NCORES = 8
GROUPS = 4  # head groups; 4 heads = 256 features per core
M = 256  # head features per core
SCALE = 0.125  # 1/sqrt(64)

# stream dtypes per matmul group: "f32r", "bf16", or "fp16"
CFG = {
    "proj": "fp16",   # xT, wq/wk/wv
    "scores": "fp16",  # QT, KT
    "av": "fp16",      # VA, exp tiles
    "wo": "fp16",      # OT, wo
}

_compiled = None


def _dt(mybir, name):
    return {"f32r": mybir.dt.float32r, "bf16": mybir.dt.bfloat16,
            "fp16": mybir.dt.float16}[name]


def _np_dt(name):
    if name == "bf16":
        import ml_dtypes
        return ml_dtypes.bfloat16
    if name == "fp16":
        return np.float16
    return np.float32


def _build_module():
    import concourse.mybir as mybir
    import concourse.tile as tile
    from concourse import bacc

    f32 = mybir.dt.float32
    in_dt = _dt(mybir, CFG["proj"])
    wo_dt = _dt(mybir, CFG["wo"])
    nc = bacc.Bacc("TRN2", target_bir_lowering=False, debug=False,
                   num_devices=NCORES)
    xT = nc.dram_tensor("xT", [D, S], in_dt, kind="ExternalInput").ap()
    wq = nc.dram_tensor("wq", [D, M], in_dt, kind="ExternalInput").ap()
    wk = nc.dram_tensor("wk", [D, M], in_dt, kind="ExternalInput").ap()
    wv = nc.dram_tensor("wv", [D, M], in_dt, kind="ExternalInput").ap()
    wo = nc.dram_tensor("wo", [M, D], wo_dt, kind="ExternalInput").ap()
    out = nc.dram_tensor("out", [S, D], f32, kind="ExternalOutput").ap()

    with tile.TileContext(nc) as tc:
        _kernel_body(tc, out, xT, wq, wk, wv, wo)
    nc.compile()
    return nc


def _kernel_body(tc, out, xT, wq, wk, wv, wo):
    from contextlib import ExitStack

    import concourse.mybir as mybir
    from concourse.masks import make_identity

    nc = tc.nc
    f32 = mybir.dt.float32
    f32r = mybir.dt.float32r
    sc_dt = _dt(mybir, CFG["scores"])
    av_dt = _dt(mybir, CFG["av"])
    wo_dt = _dt(mybir, CFG["wo"])
    AF = mybir.ActivationFunctionType

    P = 128
    NKT = D // P   # 8 k-tiles in the projection contraction
    NPT = M // P   # 2 partition-tiles of head features
    SKT = S // P   # 16 key tiles
    QC = 512       # q chunk (psum bank width in f32)
    NQC = S // QC  # 4
    HPC = 4        # heads per core

    with ExitStack() as ctx:
        const = ctx.enter_context(tc.tile_pool(name="const", bufs=1))
        big = ctx.enter_context(tc.tile_pool(name="big", bufs=1))
        wpool = ctx.enter_context(tc.tile_pool(name="w", bufs=1))
        work = ctx.enter_context(tc.tile_pool(name="work", bufs=2))
        exp_pool = ctx.enter_context(tc.tile_pool(name="exp", bufs=6))
        small = ctx.enter_context(tc.tile_pool(name="small", bufs=2))
        # PSUM budget (8 banks): psA 2 + psS 2x2 + psO 2x1 = 8
        psum_big = ctx.enter_context(tc.tile_pool(name="psA", bufs=2, space="PSUM"))
        psum_s = ctx.enter_context(tc.tile_pool(name="psS", bufs=2, space="PSUM"))
        psum_o = ctx.enter_context(tc.tile_pool(name="psO", bufs=1, space="PSUM"))

        ident_f = const.tile([P, P], f32)
        make_identity(nc, ident_f)
        ident = const.tile([P, P], f32r, tag="ident_r")
        nc.vector.tensor_copy(ident[:], ident_f[:])

        # warm the PE clock (HAM) during the input DMA head: ~40 dummy
        # matmuls on the identity keep the activity monitor busy so the
        # real projections start at 2.4GHz instead of 1.2
        warm_ps = psum_big.tile([P, P], f32, tag="ps_big")
        for _ in range(40):
            nc.tensor.matmul(warm_ps[:], ident[:], ident[:],
                             start=True, stop=True)

        QT = big.tile([P, NPT, S], sc_dt, tag="QT")
        KT = big.tile([P, NPT, S], sc_dt, tag="KT")
        # VT is fully consumed by the transpose phase before OT is first
        # written, so they share a buffer slot.
        VT = big.tile([P, NPT, S], f32r, tag="VT_OT")
        proj_dst = {"q": QT, "k": KT, "v": VT}
        wo_sb = wpool.tile([P, NPT, D], wo_dt, tag="wo")
        nc.sync.dma_start(wo_sb[:], wo.rearrange("(pt p) n -> p pt n", p=P))

        # xT and the q/k/v weights are only needed for the projections;
        # scope them so their SBUF is reusable afterwards.
        with tc.tile_pool(name="projin", bufs=1) as projin:
            # weights on the gpsimd (SWDGE) queue, bulk xT on the sync
            # (HWDGE) queue so they transfer in parallel and the first
            # projection matmul isn't stuck behind the bulk transfer
            w_sb = {}
            for name, w in (("k", wk), ("v", wv), ("q", wq)):
                t = projin.tile([P, NKT, M], w.dtype, tag=f"w{name}")
                nc.gpsimd.dma_start(t[:], w.rearrange("(kt p) m -> p kt m", p=P))
                w_sb[name] = t

            xT_sb = projin.tile([P, NKT, S], xT.dtype, tag="xT")
            xT_r = xT.rearrange("(kt p) s -> p kt s", p=P)
            for c in range(NQC):
                for kh in range(4 if c == 0 else 2):
                    n = NKT // (4 if c == 0 else 2)
                    nc.sync.dma_start(
                        xT_sb[:, kh * n:(kh + 1) * n, c * QC:(c + 1) * QC],
                        xT_r[:, kh * n:(kh + 1) * n, c * QC:(c + 1) * QC])

            # --- q/k/v projections: PT[f, s] = sum_d w[d, f] * xT[d, s].
            # pt-outer so head pair 0 is fully projected (and transposed)
            # before pair 1 starts; pair 0's attention overlaps pair 1's
            # projections. ---
            VA = big.tile([P, HPC, SKT, P], av_dt, tag="VA")
            ones_f = const.tile([P, 64], f32, tag="ones")
            nc.any.memset(ones_f[:], 1.0)
            for h in range(HPC):
                for st in range(SKT):
                    nc.vector.tensor_copy(VA[:, h, st, 64:128], ones_f[:])
            for pt in range(NPT):
                for name in ("k", "v", "q"):
                    dst = proj_dst[name]
                    for c in range(NQC):
                        ps = psum_big.tile([P, QC], f32, tag="ps_big")
                        for kt in range(NKT):
                            nc.tensor.matmul(
                                ps[:],
                                w_sb[name][:, kt, pt * P:(pt + 1) * P],
                                xT_sb[:, kt, c * QC:(c + 1) * QC],
                                start=(kt == 0), stop=(kt == NKT - 1),
                            )
                        nc.any.tensor_copy(dst[:, pt, c * QC:(c + 1) * QC], ps[:])
                # V back to natural layout + ones block for softmax sums
                for st in range(SKT):
                    pst = psum_big.tile([P, P], f32r, tag="ps_big")
                    nc.tensor.transpose(pst[:], VT[:, pt, st * P:(st + 1) * P],
                                        ident)
                    nc.any.tensor_copy(VA[:, 2 * pt, st, 0:64], pst[:, 0:64])
                    nc.any.tensor_copy(VA[:, 2 * pt + 1, st, 0:64],
                                       pst[:, 64:128])

        # 2-byte OT fits alongside VT; f32r must reuse VT's slot (SBUF)
        OT = big.tile([P, NPT, S], wo_dt,
                      tag="VT_OT" if wo_dt == f32r else "OT")

        # --- attention: a head pair per iteration; both heads' transposed
        # score tiles land in one 2-bank psum tile, one wide exp serves both,
        # and the PE always has an independent chain while ACT runs ---
        # matmul psum output must be f32; one key-tile (head pair) per fill
        KG, sc_ps_dt = 1, f32
        for p in range(NPT):
            for c in range(NQC):
                cs = slice(c * QC, (c + 1) * QC)
                poA = psum_o.tile([P, QC], f32, tag="ps_oA")
                poB = psum_o.tile([P, QC], f32, tag="ps_oB")
                for k2 in range(SKT // KG):
                    pss = psum_s.tile([P, 2 * KG, QC], sc_ps_dt, tag="ps_s")
                    for j in range(KG):
                        kt = KG * k2 + j
                        ks = slice(kt * P, (kt + 1) * P)
                        nc.tensor.matmul(pss[:, 2 * j, :], KT[0:64, p, ks],
                                         QT[0:64, p, cs], start=True, stop=True)
                        nc.tensor.matmul(pss[:, 2 * j + 1, :],
                                         KT[64:128, p, ks],
                                         QT[64:128, p, cs], start=True,
                                         stop=True)
                    et = exp_pool.tile([P, 2 * KG, QC], av_dt, tag="exp")
                    nc.scalar.activation(et[:], pss[:], AF.Exp, scale=SCALE)
                    for j in range(KG):
                        kt = KG * k2 + j
                        nc.tensor.matmul(poA[:], VA[:, 2 * p, kt, :],
                                         et[:, 2 * j, :],
                                         start=(kt == 0), stop=(kt == SKT - 1))
                        nc.tensor.matmul(poB[:], VA[:, 2 * p + 1, kt, :],
                                         et[:, 2 * j + 1, :],
                                         start=(kt == 0), stop=(kt == SKT - 1))
                for r0, po in ((0, poA), (64, poB)):
                    # evacuate po in one copy so the psum bank frees early,
                    # then normalize from the SBUF copy off the critical path
                    pc = small.tile([P, QC], f32, tag="po_sb")
                    nc.vector.tensor_copy(pc[:], po[:])
                    sm = small.tile([64, QC], f32, tag="sums")
                    nc.vector.tensor_copy(sm[:], pc[64:128, :])
                    rb = small.tile([64, QC], f32, tag="recip")
                    nc.vector.reciprocal_approx_fast(rb[:], sm[:])
                    nc.vector.tensor_tensor(
                        OT[r0:r0 + 64, p, cs],
                        pc[0:64, :],
                        rb[:],
                        mybir.AluOpType.mult,
                    )

        # --- output projection: out[s, n] = sum_f OT[f, s] * wo[f, n] ---
        for qt in range(S // P):
            for nch in range(2):
                ps = psum_big.tile([P, 512], f32, tag="ps_big")
                for pt in range(NPT):
                    nc.tensor.matmul(
                        ps[:],
                        OT[:, pt, qt * P:(qt + 1) * P],
                        wo_sb[:, pt, nch * 512:(nch + 1) * 512],
                        start=(pt == 0), stop=(pt == NPT - 1),
                    )
                ot = work.tile([P, 512], f32, tag="outstage")
                nc.any.tensor_copy(ot[:], ps[:])
                nc.sync.dma_start(
                    out[qt * P:(qt + 1) * P, nch * 512:(nch + 1) * 512], ot[:])


def _in_maps(x, Wq, Wk, Wv, Wo):
    in_np = _np_dt(CFG["proj"])
    wo_np = _np_dt(CFG["wo"])
    x = np.asarray(x, dtype=np.float32)
    Wq = np.asarray(Wq, dtype=np.float32)
    Wk = np.asarray(Wk, dtype=np.float32)
    Wv = np.asarray(Wv, dtype=np.float32)
    Wo = np.asarray(Wo, dtype=np.float32)
    xT = [np.ascontiguousarray(x[b].T).astype(in_np) for b in range(B)]
    maps = []
    for c in range(NCORES):
        b, g = c // GROUPS, c % GROUPS
        rows = slice(g * M, (g + 1) * M)
        maps.append({
            "xT": xT[b],
            "wq": np.ascontiguousarray(Wq[rows, :].T).astype(in_np),
            "wk": np.ascontiguousarray(Wk[rows, :].T).astype(in_np),
            "wv": np.ascontiguousarray(Wv[rows, :].T).astype(in_np),
            "wo": np.ascontiguousarray(Wo[:, rows].T).astype(wo_np),
        })
    return maps


def kernel(x, Wq, Wk, Wv, Wo, _trace=False):
    global _compiled
    if _compiled is None:
        _compiled = _build_module()
    from concourse.bass_utils import run_bass_kernel_spmd

    res = run_bass_kernel_spmd(
        _compiled, _in_maps(x, Wq, Wk, Wv, Wo),
        core_ids=list(range(NCORES)), trace=_trace,
    )
    outs = [r["out"] for r in res.results]
    y = np.empty((B, S, D), np.float32)
    for b in range(B):
        y[b] = outs[4 * b] + outs[4 * b + 1] + outs[4 * b + 2] + outs[4 * b + 3]
    if _trace:
        kernel.last_results = res
    return y
